# revision 20
# baseline (speedup 1.0000x reference)
"""Trainium2 Bass kernel for nn_EnhancedLocalAttention.

Reference semantics (B=4, L=4096, C=1024, H=16, D=64, WIN=256, step=128):
  qkv = x @ W_qkv + b_qkv -> q,k,v [B,H,L,D]
  overlapping windows n: tokens [n*128, n*128+256)
  per (b,h,n): S = (Q_win^T K_win)/8  (D x D, contracted over the 256 window
  tokens), P = softmax(S, axis=-1), O = P @ V_win^T  (D x W)
  regroup: rows of reshape(O, [256, 64]) laid at tokens n*256..n*256+255,
  slice to L -> only windows 0..15 survive; then @ W_out + b_out.

Sharding: 8 cores = (4 batches) x (2 window-halves of 8 windows each).
Each core consumes 9 x 128-token chunks and produces 2048 output rows.

Per-round pipeline (round r): QKV projection for chunk r (output-major,
N=512 matmuls into per-colgroup PSUM banks, drained by the scalar engine)
interleaved at unit granularity with attention + out-projection for window
r-2.  x^T and V^T come from xbar DMA-transposes (no PE transposes, no DVE
copies for them).  Softmax: exp(scale=1/8) on scalar, row-sum on DVE,
normalize on GpSimd -- engines balanced so the PE stays fed.
"""

import threading

import numpy as np

import concourse.bacc as bacc
import concourse.masks as masks
import concourse.mybir as mybir
import concourse.tile as tile
from concourse._compat import get_trn_type
from concourse.bass_utils import run_bass_kernel_spmd

F32 = mybir.dt.float32
F16 = mybir.dt.float16
EXP = mybir.ActivationFunctionType.Exp

B, L, C = 4, 4096, 1024
H, D, WIN, STEP = 16, 64, 256, 128
NCHUNK = 9            # 128-token chunks per core
NWIN = 8              # windows per core
TOK = NCHUNK * 128    # 1152 input tokens per core
OUT_ROWS = NWIN * 256 # 2048 output rows per core


def interleave(a, b):
    """Merge two unit lists proportionally (Bresenham)."""
    if not b:
        return list(a)
    if not a:
        return list(b)
    out = []
    ia = ib = 0
    while ia < len(a) or ib < len(b):
        if ib >= len(b) or (ia < len(a) and ia * len(b) <= ib * len(a)):
            out.append(a[ia]); ia += 1
        else:
            out.append(b[ib]); ib += 1
    return out


def build_program(with_bias=True):
    nc = bacc.Bacc(
        get_trn_type() or "TRN2",
        target_bir_lowering=False,
        debug=False,
        num_devices=8,
    )
    xs = nc.dram_tensor("xs", [TOK, C], F32, kind="ExternalInput")
    wqkv = nc.dram_tensor("wqkv", [C, 3 * C], F32, kind="ExternalInput")
    bqkv = nc.dram_tensor("bqkv", [3 * C], F32, kind="ExternalInput")
    bout = nc.dram_tensor("bout", [C], F32, kind="ExternalInput")
    wout = nc.dram_tensor("wout", [C, C], F32, kind="ExternalInput")
    out = nc.dram_tensor("out", [OUT_ROWS, C], F32, kind="ExternalOutput")

    from contextlib import ExitStack

    with tile.TileContext(nc) as tc, ExitStack() as ctx:
        pool = lambda name, bufs: ctx.enter_context(tc.tile_pool(name=name, bufs=bufs))
        w_pool = pool("w", 48)
        wo_pool = pool("wo", 8)
        const_pool = pool("const", 1)
        x_pool = pool("x", NCHUNK)
        xt_pool = pool("xt", 18)
        qk_pool = pool("qk", 8)
        v_pool = pool("v", 2)
        vt_pool = pool("vt", 32)
        at_pool = pool("at", 6)
        st_pool = pool("st", 8)
        yt_pool = pool("yt", 12)
        o_pool = pool("o", 3)
        ps = ctx.enter_context(tc.tile_pool(name="ps", bufs=2, space="PSUM"))

        # --- constants ---
        idf16 = const_pool.tile([128, 128], F16, tag="idf16", name="idf16")
        masks.make_identity(nc, idf16[:])
        ones = const_pool.tile([1, 128], F16, tag="ones", name="ones")
        nc.vector.memset(ones[:], 1.0)
        if with_bias:
            bq_sb = const_pool.tile([1, 3 * C], F16, tag="bq", name="bq_sb")
            nc.gpsimd.dma_start(bq_sb[:], bqkv.ap().rearrange("(a f) -> a f", a=1))
            bo_sb = const_pool.tile([1, C], F16, tag="bo", name="bo_sb")
            nc.gpsimd.dma_start(bo_sb[:], bout.ap().rearrange("(a f) -> a f", a=1))

        # --- input DMAs: all casting loads go through gpsimd (SWDGE),
        # interleaved in consumption order: x0 x1 Q0 Q1 x2 x3 K0 x4 x5 K1
        # x6 x7 V0 x8 V1 wo.  sync (HWDGE) keeps the non-casting xbar
        # transposes and output writes.
        x_pre = [None] * NCHUNK
        w_sb = [[None] * 8 for _ in range(6)]
        wo_sb = []

        def load_x(r):
            x_t = x_pool.tile([128, C], F16, tag="x", name=f"x{r}")
            nc.gpsimd.dma_start(x_t[:], xs.ap()[r * 128 : (r + 1) * 128, :])
            x_pre[r] = x_t

        def load_w(cg, cbs):
            for cb in cbs:
                t = w_pool.tile([128, 512], F16, tag="w", name=f"w{cg}_{cb}")
                nc.gpsimd.dma_start(
                    t[:],
                    wqkv.ap()[cb * 128 : (cb + 1) * 128, cg * 512 : (cg + 1) * 512],
                )
                w_sb[cg][cb] = t

        def load_wpair(pr):
            # halves interleaved so the pair's first matmuls start sooner
            load_w(2 * pr, range(4))
            load_w(2 * pr + 1, range(4))
            load_w(2 * pr, range(4, 8))
            load_w(2 * pr + 1, range(4, 8))

        xt_all = [None] * NCHUNK
        vt_sb = [[None] * 8 for _ in range(NCHUNK)]

        def transpose_x(r, fb0, n):
            """PE-transpose x chunk r, feature blocks fb0..fb0+n-1."""
            if xt_all[r] is None:
                xt_all[r] = [None] * 8
            for fb in range(fb0, fb0 + n):
                tp = ps.tile([128, 128], F16, tag="tr", bufs=2, name="tp")
                nc.tensor.transpose(
                    tp[:], x_pre[r][:, fb * 128 : (fb + 1) * 128], idf16[:]
                )
                xtt = xt_pool.tile([128, 128], F16, tag="xt", name="xtt")
                nc.vector.tensor_copy(xtt[:], tp[:])
                xt_all[r][fb] = xtt

        def load_wo(cbs):
            for cb in cbs:
                t = wo_pool.tile([128, C], F16, tag="wo", name=f"wo{cb}")
                nc.gpsimd.dma_start(t[:], wout.ap()[cb * 128 : (cb + 1) * 128, :])
                wo_sb.append(t)

        # deadline-ordered input stream on the (casting) gpsimd queue
        load_x(0)
        load_x(1)
        load_wpair(0)
        load_wpair(1)
        load_x(2)
        load_x(3)
        load_wpair(2)
        load_x(4)
        load_wo(range(8))
        load_x(5)
        load_x(6)
        load_x(7)
        load_x(8)

        q_sb = [None] * NCHUNK
        k_sb = [None] * NCHUNK

        def qkv_units(r):
            """Chunk r QKV: colgroup pairs (Q, K, V), 2 psum banks per pair,
            stationary x^T block shared by the pair's two matmuls."""
            st = {}

            def mk_A(pr):
                def f():
                    P0 = ps.tile([128, 512], F32, tag="big", bufs=4, name=f"p{pr}a")
                    P1 = ps.tile([128, 512], F32, tag="big", bufs=4, name=f"p{pr}b")
                    st[pr] = (P0, P1)
                    for cb in range(4):
                        lhs = xt_all[r][cb][:]
                        nc.tensor.matmul(
                            P0[:], lhs, w_sb[2 * pr][cb][:], start=(cb == 0), stop=False
                        )
                        nc.tensor.matmul(
                            P1[:], lhs, w_sb[2 * pr + 1][cb][:], start=(cb == 0), stop=False
                        )
                return f

            def mk_B(pr):
                def f():
                    P0, P1 = st[pr]
                    last = not with_bias
                    for cb in range(4, 8):
                        lhs = xt_all[r][cb][:]
                        nc.tensor.matmul(
                            P0[:], lhs, w_sb[2 * pr][cb][:],
                            start=False, stop=(cb == 7 and last),
                        )
                        nc.tensor.matmul(
                            P1[:], lhs, w_sb[2 * pr + 1][cb][:],
                            start=False, stop=(cb == 7 and last),
                        )
                return f

            def mk_F(pr, half):
                def f():
                    P = st[pr][half]
                    cg = 2 * pr + half
                    if with_bias:
                        nc.tensor.matmul(
                            P[:], ones[:, :], bq_sb[:, cg * 512 : (cg + 1) * 512],
                            start=False, stop=True,
                        )
                    if pr == 0:
                        if half == 0:
                            q_sb[r] = qk_pool.tile([128, C], F16, tag="q", bufs=4, name="qt")
                        dest = q_sb[r]
                    elif pr == 1:
                        if half == 0:
                            k_sb[r] = qk_pool.tile([128, C], F16, tag="k", bufs=4, name="kt")
                        dest = k_sb[r]
                    else:
                        if half == 0:
                            st["v"] = v_pool.tile([128, C], F16, tag="v", name="v_t")
                        dest = st["v"]
                    nc.scalar.copy(dest[:, half * 512 : (half + 1) * 512], P[:])
                return f

            def mk_vt(fb0):
                def f():
                    for fb in (fb0, fb0 + 1):
                        tpv = ps.tile([128, 128], F16, tag="tr", bufs=2, name="tpv")
                        nc.tensor.transpose(
                            tpv[:], st["v"][:, fb * 128 : (fb + 1) * 128], idf16[:]
                        )
                        vtt = vt_pool.tile([128, 128], F16, tag="vt", name="vtt")
                        nc.scalar.copy(vtt[:], tpv[:])
                        vt_sb[r][fb] = vtt
                return f

            units = []
            if r == 0:
                units += [lambda: transpose_x(0, 0, 4), lambda: transpose_x(0, 4, 4)]
            for pr in range(3):
                units += [mk_A(pr), mk_B(pr), mk_F(pr, 0), mk_F(pr, 1)]
            units += [mk_vt(0), mk_vt(2), mk_vt(4), mk_vt(6)]
            if r + 1 < NCHUNK:
                units.insert(6, lambda: transpose_x(r + 1, 0, 4))
                units.insert(10, lambda: transpose_x(r + 1, 4, 4))
            return units

        def window_units(w):
            """Window w (chunks w, w+1): S+softmax, P^T+O^T, out-proj."""
            yt = [None] * 8
            hps = [{} for _ in range(8)]
            s_store = [None, None]
            y_store = [None] * 4
            op_store = {}

            def u_hp_s(hp):
                def f():
                    if hp % 4 == 0:
                        s_store[hp // 4] = ps.tile(
                            [128, 512], F32, tag="sy", bufs=2, name="s4"
                        )
                    sq = s_store[hp // 4][:, (hp % 4) * 128 : (hp % 4 + 1) * 128]
                    for rr, (b0, b1) in ((w, (True, False)), (w + 1, (False, True))):
                        nc.tensor.matmul(
                            sq,
                            q_sb[rr][:, hp * 128 : (hp + 1) * 128],
                            k_sb[rr][:, hp * 128 : (hp + 1) * 128],
                            start=b0, stop=b1,
                        )
                    p_exp = at_pool.tile([128, 64], F16, tag="p_exp", name="p_exp")
                    nc.scalar.activation(p_exp[0:64, :], sq[0:64, 0:64], EXP, scale=0.125)
                    nc.scalar.activation(
                        p_exp[64:128, :], sq[64:128, 64:128], EXP, scale=0.125
                    )
                    ssum = st_pool.tile([128, 1], F32, tag="ssum", name="ssum")
                    nc.vector.tensor_reduce(
                        ssum[:], p_exp[:], mybir.AxisListType.X, mybir.AluOpType.add
                    )
                    rs = st_pool.tile([128, 1], F32, tag="rs", name="rs")
                    nc.vector.reciprocal(rs[:], ssum[:])
                    p_n = at_pool.tile([128, 64], F16, tag="p_n", name="p_n")
                    nc.vector.tensor_scalar_mul(p_n[:], p_exp[:], rs[:])
                    hps[hp]["p_n"] = p_n
                return f

            def u_hp_o(hp):
                def f():
                    p_n = hps[hp]["p_n"]
                    ptp = ps.tile([128, 64], F16, tag="tr", bufs=2, name="ptp")
                    nc.tensor.transpose(ptp[0:64, :], p_n[0:64, :], idf16[0:64, 0:64])
                    nc.tensor.transpose(
                        ptp[64:128, :], p_n[64:128, :], idf16[64:128, 64:128]
                    )
                    ptsb = at_pool.tile([128, 64], F16, tag="ptsb", name="ptsb")
                    nc.vector.tensor_copy(ptsb[:], ptp[:])

                    if hp % 2 == 0:
                        y_store[hp // 2] = ps.tile(
                            [128, 512], F32, tag="sy", bufs=2, name="ypair"
                        )
                    yp = y_store[hp // 2][:, (hp % 2) * 256 : (hp % 2) * 256 + 256]
                    for hip in (0, 1):
                        po = hip * 64
                        rh = ptsb[po : po + 64, :]
                        for wq in range(4):
                            vtt = vt_sb[w + wq // 2][hp]
                            nc.tensor.matmul(
                                yp[po : po + 64, wq * 64 : (wq + 1) * 64],
                                vtt[po : po + 64, (wq % 2) * 64 : (wq % 2) * 64 + 64],
                                rh,
                                start=True, stop=True,
                            )
                    ytt = yt_pool.tile([128, 256], F16, tag="yt", name="ytt")
                    # Y^T[c, d*4+wq] = yp[c, wq*64+d]  (torch-unfold regroup)
                    nc.vector.tensor_copy(
                        ytt[:].rearrange("p (b a) -> p a b", a=4),
                        yp.rearrange("p (a b) -> p a b", a=4),
                    )
                    yt[hp] = ytt
                return f

            def mk_P1(th):
                def f():
                    po_m = [
                        ps.tile([128, 512], F32, tag="big", bufs=4, name=f"pom{i}")
                        for i in range(2)
                    ]
                    op_store[th] = po_m
                    for cb in range(4):
                        for mi in range(2):
                            nc.tensor.matmul(
                                po_m[mi][:],
                                yt[cb][:, th * 128 : (th + 1) * 128],
                                wo_sb[cb][:, mi * 512 : (mi + 1) * 512],
                                start=(cb == 0), stop=False,
                            )
                return f

            def mk_P2(th):
                def f():
                    po_m = op_store[th]
                    for cb in range(4, 8):
                        for mi in range(2):
                            nc.tensor.matmul(
                                po_m[mi][:],
                                yt[cb][:, th * 128 : (th + 1) * 128],
                                wo_sb[cb][:, mi * 512 : (mi + 1) * 512],
                                start=False, stop=(cb == 7 and not with_bias),
                            )
                return f

            def mk_P3(th):
                def f():
                    po_m = op_store[th]
                    if with_bias:
                        for mi in range(2):
                            nc.tensor.matmul(
                                po_m[mi][:],
                                ones[:, :], bo_sb[:, mi * 512 : (mi + 1) * 512],
                                start=False, stop=True,
                            )
                    ot = o_pool.tile([128, C], F32, tag="o", name="ot")
                    nc.vector.tensor_copy(ot[:, 0:512], po_m[0][:])
                    nc.vector.tensor_copy(ot[:, 512:1024], po_m[1][:])
                    row = w * 256 + th * 128
                    nc.sync.dma_start(out.ap()[row : row + 128, :], ot[:])
                return f

            units = [u_hp_s(0), u_hp_s(1), u_hp_s(2)]
            for hp in range(3, 8):
                units += [u_hp_s(hp), u_hp_o(hp - 3)]
            units += [u_hp_o(5), u_hp_o(6), u_hp_o(7)]
            units += [mk_P1(0), mk_P2(0), mk_P3(0), mk_P1(1), mk_P2(1), mk_P3(1)]
            return units

        for r in range(NCHUNK + 1):
            qk = qkv_units(r) if r < NCHUNK else []
            win = window_units(r - 2) if 2 <= r < NWIN + 2 else []
            for u in interleave(qk, win):
                u()

    nc.compile()
    return nc


_CACHE = {}
_LOCK = threading.Lock()


def _get_program(with_bias=True):
    key = f"nc_bias{with_bias}"
    with _LOCK:
        if key not in _CACHE:
            _CACHE[key] = build_program(with_bias=with_bias)
        return _CACHE[key]


def kernel(x, W_qkv, b_qkv, W_out, b_out):
    x = np.asarray(x, dtype=np.float32)
    W_qkv = np.asarray(W_qkv, dtype=np.float32)
    b_qkv = np.asarray(b_qkv, dtype=np.float32)
    W_out = np.asarray(W_out, dtype=np.float32)
    b_out = np.asarray(b_out, dtype=np.float32)

    with_bias = bool(np.any(b_qkv)) or bool(np.any(b_out))
    nc = _get_program(with_bias=with_bias)
    in_maps = []
    for cid in range(8):
        b, half = cid // 2, cid % 2
        t0 = half * NWIN * STEP
        in_maps.append(
            {
                "xs": np.ascontiguousarray(x[b, t0 : t0 + TOK, :]),
                "wqkv": W_qkv,
                "bqkv": b_qkv,
                "wout": W_out,
                "bout": b_out,
            }
        )
    res = run_bass_kernel_spmd(nc, in_maps, core_ids=list(range(8)))
    out_full = np.empty((B, L, C), dtype=np.float32)
    for cid in range(8):
        b, half = cid // 2, cid % 2
        out_full[b, half * OUT_ROWS : (half + 1) * OUT_ROWS, :] = res.results[cid][
            "out"
        ]
    return out_full
